# revision 88
# baseline (speedup 1.0000x reference)
"""MI-LSTM full-forward Trainium2 kernel (8 NeuronCores, data-parallel batch).

Entire model runs on device per core (batch shard of 256):
  stage 1: 21 shared-weight scalar-input LSTMs, fused over (series, batch)
           = 5376 sequences/core, 50 steps, state transposed [H, seq].
  stage 2: MI-LSTM with 3-branch input attention, x-projections fused into
           the stage-1 step (block matmuls over the 21 series' h outputs).
  stage 3: temporal attention over T + dense head, via PE transposes into a
           [batch, (t, h)] layout so softmax-over-T is free-dim math.

Numerics: fp16 data / fp32 psum. All sigmoids are tanh-basis
(sig(x) = (tanh(x/2)+1)/2) so every activation instr is Tanh(scale=0.5)
and Exp stays in the same ACT table set. Doubling folds (h stored as 2h,
c as 2c) keep the +1 shifts free; compensating 1/2 factors are folded
into the weights host-side.
"""
import os
import sys

sys.path.insert(0, "/opt/trn_rl_repo")

import numpy as np

H = 64
NS = 10
S = 21
B = 2048
T = 50
NCORES = 8
BC = B // NCORES          # 256
N = S * BC                # 5376 sequences per core
HN = N // 2               # 2688
DP = NS * H

_CACHE = {}
_LAST_HW_NS = None


def _build(nc_mod, tile_mod, bacc_mod, mybir):
    f16 = mybir.dt.float16
    f32 = mybir.dt.float32
    AX = mybir.AxisListType
    OP = mybir.AluOpType
    AF = mybir.ActivationFunctionType
    from contextlib import ExitStack

    nc = bacc_mod.Bacc("TRN2", target_bir_lowering=False, debug=False,
                       num_devices=NCORES)

    dram = {}
    def din(name, shape, dt=f16):
        dram[name] = nc.dram_tensor(name, shape, dt, kind="ExternalInput").ap()
    din("xT", [T, N])
    din("k1e", [66, 256])
    din("wzx", [66, 22 * 128])
    din("whh", [64, 512])
    din("waa", [64, 64])
    din("wtb50", [128, T * 64])
    din("idn", [128, 128])
    din("wd1", [64, 64])
    din("wd2", [64, 1])
    din("bd1", [64, 1], f32)
    din("btp", [128, 1], f32)
    din("dk", [3, 192])
    din("cst", [1, 4], f32)   # [bt, bd2, unused, unused]
    din("chain", [1, 4], f32)
    out_ap = nc.dram_tensor("out", [1, BC], f32, kind="ExternalOutput").ap()
    chout_ap = nc.dram_tensor("chout", [1, 4], f32, kind="ExternalOutput").ap()
    DBG = os.environ.get("KERNEL_DEBUG", "0") == "1"
    dbg = {}
    if DBG:
        for nm, shp in [("d_g1", [128, N]), ("d_h0", [64, N]),
                        ("d_cc0", [128, HN]), ("d_tt0", [128, 1024]),
                        ("d_e1", [3, BC]), ("d_l2c1", [64, BC]),
                        ("d_h2all", [64, T * BC]), ("d_std", [128, T * 64]),
                        ("d_ctxT", [64, 256]), ("d_lg1", [64, 768]),
                        ("d_gs31", [64, 768]), ("d_lt1", [64, 768]),
                        ("d_tt1", [128, 1024]), ("d_gst1", [64, BC]),
                        ("d_cc2in1", [64, BC]), ("d_h1", [64, N])]:
            dbg[nm] = nc.dram_tensor(nm, shp, mybir.dt.float16
                                     if nm != "d_ctxT" else mybir.dt.float16,
                                     kind="ExternalOutput").ap()

    with tile_mod.TileContext(nc) as tc:
        with ExitStack() as ctx:
            wp = ctx.enter_context(tc.tile_pool(name="wp", bufs=1))
            sp = ctx.enter_context(tc.tile_pool(name="sp", bufs=1))
            ps1 = ctx.enter_context(tc.tile_pool(name="ps1", bufs=2, space="PSUM"))
            ps2 = ctx.enter_context(tc.tile_pool(name="ps2", bufs=1, space="PSUM"))

            # ---- weights ----
            k1e = wp.tile([66, 256], f16)
            wzx = wp.tile([66, 22 * 128], f16)
            whh = wp.tile([64, 512], f16)
            waa = wp.tile([64, 64], f16)
            wtb50 = wp.tile([128, T * 64], f16)
            idn = wp.tile([128, 128], f16)
            wd1 = wp.tile([64, 64], f16)
            wd2 = wp.tile([64, 1], f16)
            bd1 = wp.tile([64, 1], f32)
            btp = wp.tile([128, 1], f32)
            dk = wp.tile([3, 192], f16)
            cst = wp.tile([1, 4], f32)
            for nm, t_ in [("k1e", k1e), ("wzx", wzx), ("whh", whh),
                           ("waa", waa), ("wtb50", wtb50), ("idn", idn),
                           ("wd1", wd1), ("wd2", wd2), ("bd1", bd1),
                           ("btp", btp), ("dk", dk), ("cst", cst)]:
                nc.sync.dma_start(t_[:], dram[nm])

            # small constant lhsTs built on device
            onesk = wp.tile([64, 9], f16)      # 3 blocks of [64,3], col k of block k = 1
            nc.vector.memset(onesk[:], 0.0)
            for k in range(3):
                nc.vector.memset(onesk[:, 3 * k + k: 3 * k + k + 1], 1.0)
            ones3 = wp.tile([3, 1], f16)
            nc.vector.memset(ones3[:], 1.0)
            ones1 = wp.tile([1, 64], f16)
            nc.vector.memset(ones1[:], 1.0)
            ones1b = wp.tile([1, 128], f16)
            nc.vector.memset(ones1b[:], 1.0)
            bneg = wp.tile([3, 1], f32)
            nc.vector.memset(bneg[:], -2.0)

            # ---- state tiles ----
            # hx rows: [H (0:64); ones (64); x_t (65)]
            hx = [sp.tile([66, N], f16, name=f"hx{i}") for i in range(2)]
            for i in range(2):
                nc.vector.memset(hx[i][0:64, :], 0.0)
                nc.vector.memset(hx[i][64:65, :], 1.0)
            nc.sync.dma_start(hx[0][65:66, :], dram["xT"][0:1, :])

            ccp = [sp.tile([128, HN], f16, name=f"ccp{i}") for i in range(2)]
            nc.vector.memset(ccp[0][:], 0.0)
            cc2 = [sp.tile([64, BC], f16, name=f"cc2{i}") for i in range(2)]
            nc.vector.memset(cc2[0][:], 0.0)
            gst = sp.tile([64, BC], f16)
            gs3 = sp.tile([64, 768], f16)
            nc.vector.memset(gs3[:], 0.0)

            gall = sp.tile([128, 2 * N], f16)
            g1f = sp.tile([128, HN], f16)
            g1i = sp.tile([128, HN], f16)
            g2o = sp.tile([128, HN], f16)
            g2j = sp.tile([128, HN], f16)
            ta = sp.tile([128, HN], f16)
            tb = sp.tile([128, HN], f16)
            thp = sp.tile([128, HN], f16)

            h2all = sp.tile([64, T * BC], f16)
            h2stdA = sp.tile([128, T * 64], f16)
            h2stdB = sp.tile([128, T * 64], f16)
            ttile = sp.tile([128, 1024], f16)
            ttp = sp.tile([128, 768], f16)
            ltile = sp.tile([64, 768], f16)
            wtl = sp.tile([128, 1024], f16)
            lg = sp.tile([64, 768], f16)
            etile = sp.tile([3, BC], f16)
            rt = sp.tile([1, BC], f16)
            l2c = sp.tile([64, BC], f16)
            tf1 = sp.tile([64, BC], f16)
            o1 = sp.tile([64, BC], f16)
            m2 = sp.tile([64, BC], f16)
            th2 = sp.tile([64, BC], f16)

            CH = [(0, 1024), (1024, 1024), (2048, 1024), (3072, 1024),
                  (4096, 1024), (5120, 256)]

            for t in range(T):
                cur, nxt = hx[t % 2], hx[(t + 1) % 2]
                ccc, ccn = ccp[t % 2], ccp[(t + 1) % 2]
                c2c, c2n = cc2[t % 2], cc2[(t + 1) % 2]

                if t + 1 < T:
                    nc.sync.dma_start(nxt[65:66, :], dram["xT"][t + 1:t + 2, :])

                # ---- stage 1: gates ----
                # gall cols 0:N = M0 ([f|i]), N:2N = M1 ([o|j]); one tanh act
                # per psum chunk (scale/bias folds live in the weights).
                # M0 first so the f/i packs and the TB product (pool) can
                # start while M1 chunks are still in the matmul/act pipe.
                for m in (0, 1):
                    for c0, cw in CH:
                        ps = ps1.tile([128, 1024], f32, tag="s1", name="s1ps")
                        for s0 in range(0, cw, 512):
                            sw = min(512, cw - s0)
                            nc.tensor.matmul(
                                ps[:, s0:s0 + sw],
                                k1e[:, m * 128:(m + 1) * 128],
                                cur[0:66, c0 + s0:c0 + s0 + sw],
                                start=True, stop=True)
                        nc.scalar.activation(
                            gall[:, N * m + c0:N * m + c0 + cw], ps[:, 0:cw],
                            AF.Tanh, bias=0.0, scale=0.5)

                # ---- stage 1: elementwise (fp16, packed halves) ----
                # gall cols 0:N = M0 tile [tf|ti], cols N:2N = M1 tile [to|tj]
                for hh in (0, 1):
                    for qq in (0, 1):
                        a = HN * hh + (HN // 2) * qq
                        b = a + HN // 2
                        sl = slice(64 * hh, 64 * (hh + 1))
                        cs = slice((HN // 2) * qq, (HN // 2) * (qq + 1))
                        nc.vector.tensor_scalar(g1f[sl, cs], gall[0:64, a:b],
                                                1.0, 0.5, OP.add, OP.mult)
                        nc.vector.tensor_scalar(g1i[sl, cs], gall[64:128, a:b],
                                                1.0, None, OP.add)
                        nc.vector.tensor_scalar(g2o[sl, cs],
                                                gall[0:64, N + a:N + b],
                                                1.0, None, OP.add)
                        nc.vector.tensor_scalar(g2j[sl, cs],
                                                gall[64:128, N + a:N + b],
                                                1.0, None, OP.mult)
                QW = HN // 4
                for q in range(4):
                    cs = slice(QW * q, QW * (q + 1))
                    nc.vector.tensor_tensor(ta[:, cs], g1i[:, cs], g2j[:, cs],
                                            OP.mult)
                    nc.gpsimd.tensor_tensor(tb[:, cs], g1f[:, cs], ccc[:, cs],
                                            OP.mult)
                    nc.vector.tensor_tensor(ccn[:, cs], ta[:, cs], tb[:, cs],
                                            OP.add)
                    nc.scalar.activation(thp[:, cs], ccn[:, cs], AF.Tanh,
                                         bias=0.0, scale=0.5)
                    for hh in (0, 1):
                        sl = slice(64 * hh, 64 * (hh + 1))
                        nc.vector.tensor_tensor(
                            nxt[0:64, HN * hh + QW * q:HN * hh + QW * (q + 1)],
                            g2o[sl, cs], thp[sl, cs], OP.mult)
                if DBG and t == 0:
                    nc.sync.dma_start(dbg["d_g1"], gall[:, 0:N])
                    nc.sync.dma_start(dbg["d_h0"], nxt[0:64, :])
                    nc.sync.dma_start(dbg["d_cc0"], ccn[:])

                # ---- stage 2: x-projections + recurrent into psum main ----
                pm = ps2.tile([128, 1024], f32, tag="s2m", name="s2m")
                # region A [0:256] = [i1|C1] over P series s=1..10
                # region B [256:512] = [i2|C2] over N series s=11..20
                # region C [512:768] = [i0|C0] over series 0
                # region D [768:1024] = [o|f] over series 0
                # each region's accumulation group must close before the next
                # region in the same psum bank opens (start zeroes the bank's
                # pending-group region)
                hprev = h2all[:, BC * (t - 1):BC * t] if t > 0 else None
                blk = 0
                for reg, srs in ((0, range(1, 11)), (1, range(11, 21)),
                                 (2, range(0, 1)), (3, range(0, 1))):
                    nsr = len(srs)
                    for j, s in enumerate(srs):
                        nc.tensor.matmul(
                            pm[:, 256 * reg:256 * (reg + 1)],
                            wzx[0:65, 128 * blk:128 * (blk + 1)],
                            nxt[0:65, BC * s:BC * (s + 1)],
                            start=(j == 0), stop=(j == nsr - 1 and t == 0))
                        blk += 1
                    if t > 0:
                        nc.tensor.matmul(pm[:, 256 * reg:256 * (reg + 1)],
                                         whh[:, 128 * reg:128 * (reg + 1)],
                                         hprev, start=False, stop=True)

                pa = ps2.tile([128, 512], f32, tag="smA", name="smA")
                if t > 0:
                    nc.tensor.matmul(pa[64:128, 0:BC], waa[:], c2c[:],
                                     start=True, stop=True)

                nc.scalar.activation(ttile[:], pm[:], AF.Tanh, bias=0.0,
                                     scale=0.5)
                if t > 0:
                    nc.scalar.activation(gst[:], pa[64:128, 0:BC], AF.Tanh,
                                         bias=0.0, scale=0.5)
                    for k in range(3):
                        nc.gpsimd.tensor_scalar(gs3[:, 256 * k:256 * (k + 1)],
                                                gst[:], 1.0, None, OP.mult)

                # l_k = (ti_k+1)*tC_k ; blocks [l1 | l2 | l0]
                nc.vector.tensor_scalar(ttp[64:128, :], ttile[0:64, 0:768],
                                        1.0, None, OP.add)
                nc.vector.tensor_tensor(ltile[:], ttp[64:128, :],
                                        ttile[64:128, 0:768], OP.mult)
                nc.vector.tensor_tensor(lg[:], ltile[:], gs3[:], OP.mult)

                # u_k into psum rows 0:3 via sparse ones-column lhsTs
                for k in range(3):
                    nc.tensor.matmul(pa[0:3, 0:BC], onesk[:, 3 * k:3 * k + 3],
                                     lg[:, BC * k:BC * (k + 1)],
                                     start=(k == 0), stop=(k == 2),
                                     skip_group_check=True)
                nc.scalar.activation(etile[:], pa[0:3, 0:BC], AF.Exp,
                                     bias=bneg[:], scale=0.5)
                nc.tensor.matmul(pa[32:33, 256:512], ones3[:], etile[:],
                                 start=True, stop=True)
                with nc.allow_low_precision(reason="softmax denom fp16"):
                    nc.vector.reciprocal(rt[:], pa[32:33, 256:512])
                nc.tensor.matmul(pa[64:128, 256:512], ones1[:], rt[:],
                                 start=True, stop=True)
                pb = ps2.tile([128, 512], f32, tag="smB", name="smB")
                nc.tensor.matmul(pb[0:64, 0:256], dk[:, 0:64], etile[:],
                                 start=True, stop=True)
                nc.tensor.matmul(pb[0:64, 256:512], dk[:, 64:128], etile[:],
                                 start=True, stop=True)
                nc.tensor.matmul(pa[0:64, 256:512], dk[:, 128:192], etile[:],
                                 start=True, stop=True)
                # W = sum_k eB_k * l_k  (rows 64:127 so bases pair with rB)
                nc.vector.tensor_tensor(wtl[64:128, 0:256], pb[0:64, 0:256],
                                        ltile[:, 0:256], OP.mult)
                nc.vector.tensor_tensor(wtl[64:128, 256:512], pb[0:64, 256:512],
                                        ltile[:, 256:512], OP.mult)
                nc.vector.tensor_tensor(wtl[64:128, 512:768], pa[0:64, 256:512],
                                        ltile[:, 512:768], OP.mult)
                # (eB/rB reads stay on DVE: gpsimd cannot access PSUM)
                nc.gpsimd.tensor_tensor(wtl[64:128, 768:1024],
                                        wtl[64:128, 0:256],
                                        wtl[64:128, 256:512], OP.add)
                nc.gpsimd.tensor_tensor(wtl[64:128, 0:256],
                                        wtl[64:128, 768:1024],
                                        wtl[64:128, 512:768], OP.add)
                nc.vector.tensor_tensor(l2c[:], wtl[64:128, 0:256],
                                        pa[64:128, 256:512], OP.mult)
                if DBG and t == 0:
                    nc.sync.dma_start(dbg["d_tt0"], ttile[:])
                if DBG and t == 1:
                    nc.sync.dma_start(dbg["d_e1"], etile[:])
                    nc.sync.dma_start(dbg["d_l2c1"], l2c[:])
                    nc.sync.dma_start(dbg["d_lg1"], lg[:])
                    nc.sync.dma_start(dbg["d_gs31"], gs3[:])
                    nc.sync.dma_start(dbg["d_lt1"], ltile[:])
                    nc.sync.dma_start(dbg["d_tt1"], ttile[:])
                    nc.sync.dma_start(dbg["d_gst1"], gst[:])
                    nc.sync.dma_start(dbg["d_cc2in1"], c2c[:])
                    nc.sync.dma_start(dbg["d_h1"], nxt[0:64, :])

                # cell + output gate
                nc.gpsimd.tensor_scalar(tf1[:], ttile[64:128, 768:1024], 1.0,
                                        0.5, OP.add, OP.mult)
                nc.gpsimd.tensor_tensor(m2[:], tf1[:], c2c[:], OP.mult)
                nc.vector.tensor_tensor(c2n[:], m2[:], l2c[:], OP.add)
                nc.scalar.activation(th2[:], c2n[:], AF.Tanh, bias=0.0,
                                     scale=0.5)
                nc.gpsimd.tensor_scalar(o1[:], ttile[0:64, 768:1024], 1.0,
                                        None, OP.add)
                nc.vector.tensor_tensor(h2all[:, BC * t:BC * (t + 1)], o1[:],
                                        th2[:], OP.mult)

                # transpose h2_t into [b, (t, h)] std layout
                pt = ps2.tile([128, 1024], f16, tag="smB", name="tp")
                for hh, dst in ((0, h2stdA), (1, h2stdB)):
                    nc.tensor.transpose(
                        pt[:, 64 * hh:64 * (hh + 1)],
                        h2all[:, BC * t + 128 * hh:BC * t + 128 * (hh + 1)],
                        idn[0:64, 0:64])
                    nc.vector.tensor_scalar(
                        dst[:, 64 * t:64 * (t + 1)],
                        pt[:, 64 * hh:64 * (hh + 1)],
                        1.0, None, OP.mult)

            if DBG:
                nc.sync.dma_start(dbg["d_h2all"], h2all[:])
                nc.sync.dma_start(dbg["d_std"], h2stdA[:])

            # ---- stage 3: temporal attention + dense head ----
            prodE = sp.tile([128, T * 64], f16)
            ctxp = sp.tile([128, T * 64], f16)
            ctx16 = sp.tile([128, 64], f16)
            ctxT = sp.tile([64, 2 * 128], f16)
            outs = sp.tile([1, BC], f32)
            for hh, std in ((0, h2stdA), (1, h2stdB)):
                ered = sp.tile([128, T], f32, name=f"ered{hh}")
                eh = sp.tile([128, T], f16, name=f"eh{hh}")
                ex = sp.tile([128, T], f16, name=f"ex{hh}")
                srd = sp.tile([128, 1], f32, name=f"srd{hh}")
                rrd = sp.tile([128, 1], f32, name=f"rrd{hh}")
                beta = sp.tile([128, T], f32, name=f"beta{hh}")
                ctxr = sp.tile([128, 64], f32, name=f"ctxr{hh}")
                nc.vector.tensor_tensor(prodE[:], std[:], wtb50[:], OP.mult)
                nc.vector.tensor_reduce(
                    ered[:], prodE[:].rearrange("p (t h) -> p t h", t=T),
                    AX.X, OP.add)
                nc.scalar.activation(eh[:], ered[:], AF.Tanh, bias=btp[:],
                                     scale=0.5)
                nc.scalar.activation(ex[:], eh[:], AF.Exp, bias=0.0, scale=1.0)
                nc.vector.tensor_reduce(srd[:], ex[:], AX.X, OP.add)
                nc.vector.reciprocal(rrd[:], srd[:])
                nc.vector.tensor_scalar(beta[:], ex[:], rrd[:], None, OP.mult)
                for tt_ in range(T):
                    nc.vector.tensor_scalar(
                        ctxp[:, 64 * tt_:64 * (tt_ + 1)],
                        std[:, 64 * tt_:64 * (tt_ + 1)],
                        beta[:, tt_:tt_ + 1], None, OP.mult)
                nc.vector.tensor_reduce(
                    ctxr[:], ctxp[:].rearrange("p (t h) -> p h t", t=T),
                    AX.X, OP.add)
                nc.vector.tensor_scalar(ctx16[:], ctxr[:], 1.0, None, OP.mult)
                ptc = ps2.tile([128, 1024], f16, tag="smB", name=f"ptc{hh}")
                nc.tensor.transpose(ptc[0:64, 0:128], ctx16[:], idn[:])
                nc.vector.tensor_scalar(ctxT[:, 128 * hh:128 * (hh + 1)],
                                        ptc[0:64, 0:128],
                                        1.0, None, OP.mult)
            if DBG:
                nc.sync.dma_start(dbg["d_ctxT"], ctxT[:])
            pz = ps2.tile([128, 1024], f32, tag="s2m", name="pz")
            nc.tensor.matmul(pz[0:64, 0:256], wd1[:], ctxT[:],
                             start=True, stop=True)
            r1 = sp.tile([64, BC], f16)
            nc.scalar.activation(r1[:], pz[0:64, 0:256], AF.Relu,
                                 bias=bd1[:], scale=1.0)
            nc.tensor.matmul(pz[0:1, 512:768], wd2[:], r1[:],
                             start=True, stop=True)
            nc.vector.tensor_scalar(outs[:], pz[0:1, 512:768],
                                    cst[0:1, 1:2], None, OP.add)
            nc.sync.dma_start(out_ap, outs[:])
            chs = sp.tile([1, 4], f32)
            nc.sync.dma_start(chs[:], dram["chain"])
            nc.sync.dma_start(chout_ap, chs[:])

    nc.compile()
    return nc


def _fold_weights(K1, b1, Wc0, bc0, Wc1, bc1, Wc2, bc2,
                  Wi0, bi0, Wi1, bi1, Wi2, bi2, Wf, bf, Wo, bo, Wa,
                  Wt, bt, Wd1, bd1, Wd2, bd2):
    f32 = np.float32
    # ---- stage-1 K1e [66, 256]: device gate order [f | i | o | j] ----
    K1 = np.asarray(K1, f32)
    b1 = np.asarray(b1, f32)
    kx, kh = K1[0], K1[1:]          # [256], [64, 256]
    gi, gj, gf, go = (slice(0, 64), slice(64, 128), slice(128, 192),
                      slice(192, 256))
    # rows: [Kh/2 (0:64); bias (64, pairs with ones row); Kx (65, x row)]
    k1e = np.zeros((66, 256), f32)
    for dcol, scol in ((0, gf), (64, gi), (128, go), (192, gj)):
        k1e[0:64, dcol:dcol + 64] = kh[:, scol] * 0.5   # h stored doubled
        k1e[64, dcol:dcol + 64] = b1[scol]
        k1e[65, dcol:dcol + 64] = kx[scol]
    k1e[64, 0:64] += 1.0                                 # forget bias
    k1e[:, 192:256] *= 2.0                               # j: tanh scale fold

    # ---- stage-2 zx blocks ----
    defs = {
        "i0": (np.asarray(Wi0, f32), np.asarray(bi0, f32), 0, H, 1.0),
        "i1": (np.asarray(Wi1, f32), np.asarray(bi1, f32), H, DP, 1.0),
        "i2": (np.asarray(Wi2, f32), np.asarray(bi2, f32), H + DP, DP, 1.0),
        "C0": (np.asarray(Wc0, f32), np.asarray(bc0, f32), 0, H, 2.0),
        "C1": (np.asarray(Wc1, f32), np.asarray(bc1, f32), H, DP, 2.0),
        "C2": (np.asarray(Wc2, f32), np.asarray(bc2, f32), H + DP, DP, 2.0),
        "f": (np.asarray(Wf, f32), np.asarray(bf, f32), 0, H, 1.0),
        "o": (np.asarray(Wo, f32), np.asarray(bo, f32), 0, H, 1.0),
    }
    # X feature s-block for gate g at series s: rows (s_rel*64:(s_rel+1)*64)
    # wzx blocks: A: s=1..10 [i1|C1]; B: s=11..20 [i2|C2]; C: s=0 [i0|C0];
    # D: s=0 [o|f].  lhsT [66, 128]: rows 0:64 h-part (/2), row 64 = 0,
    # row 65 = bias (first block of each region only, scaled like gate).
    wzx = np.zeros((66, 22 * 128), f32)
    blk = 0
    regions = [("i1", "C1", list(range(1, 11))),
               ("i2", "C2", list(range(11, 21))),
               ("i0", "C0", [0]), ("o", "f", [0])]
    for gl, gr, srs in regions:
        for j, s in enumerate(srs):
            for half, gname in ((0, gl), (1, gr)):
                Wg, bg, x0, xd, sc = defs[gname]
                s_rel = s - (1 if gname in ("i1", "C1") else
                             11 if gname in ("i2", "C2") else 0)
                rows = Wg[s_rel * 64: s_rel * 64 + 64]  # x-part block [64, 64]
                wzx[0:64, 128 * blk + 64 * half:128 * blk + 64 * (half + 1)] \
                    = rows * 0.5 * sc
                if j == 0:
                    wzx[64, 128 * blk + 64 * half:
                        128 * blk + 64 * (half + 1)] = bg * sc
            blk += 1

    whh = np.zeros((64, 512), f32)
    for reg, (gl, gr) in enumerate((("i1", "C1"), ("i2", "C2"),
                                    ("i0", "C0"), ("o", "f"))):
        for half, gname in ((0, gl), (1, gr)):
            Wg, bg, x0, xd, sc = defs[gname]
            whh[:, 128 * reg + 64 * half:128 * reg + 64 * (half + 1)] = \
                Wg[xd:] * 0.5 * sc

    waa = np.asarray(Wa, f32)                           # unscaled: CC2 = 2c
    wtb50 = np.tile(np.asarray(Wt, f32).reshape(1, 1, 64),
                    (128, T, 1)).reshape(128, T * 64)
    wd1 = np.asarray(Wd1, f32) * 0.5                    # ctx doubled
    wd2 = np.asarray(Wd2, f32).reshape(64, 1)
    bd1v = np.asarray(bd1, f32).reshape(64, 1)
    btf = float(np.asarray(bt).reshape(-1)[0])
    cstv = np.array([[btf, float(np.asarray(bd2).reshape(-1)[0]), 0.0, 0.0]],
                    f32)
    f16 = np.float16
    return dict(k1e=k1e.astype(f16), wzx=wzx.astype(f16),
                whh=whh.astype(f16), waa=waa.astype(f16),
                wtb50=wtb50.astype(f16),
                idn=np.eye(128, dtype=f16), wd1=wd1.astype(f16),
                wd2=wd2.astype(f16), bd1=bd1v,
                btp=np.full((128, 1), btf, f32),
                dk=np.kron(np.eye(3), np.ones((1, 64))).astype(f16), cst=cstv)


def _timed_spmd(nc, in_maps, reps=6):
    """Replicate bass2jax.run_bass_via_pjrt's multi-core path, but keep the
    compiled callable and device-resident inputs so repeated executions
    measure steady-state device time (dispatch overhead included)."""
    import time
    import jax
    import numpy as jnp_np
    from jax.sharding import Mesh, PartitionSpec, NamedSharding
    from jax.experimental.shard_map import shard_map
    from concourse import mybir
    from concourse.bass2jax import (install_neuronx_cc_hook, _bass_exec_p,
                                    partition_id_tensor)

    install_neuronx_cc_hook()
    n_cores = len(in_maps)
    partition_name = (nc.partition_id_tensor.name
                      if nc.partition_id_tensor else None)
    in_names, out_names, out_avals, zero_outs = [], [], [], []
    for alloc in nc.m.functions[0].allocations:
        if not isinstance(alloc, mybir.MemoryLocationSet):
            continue
        name = alloc.memorylocations[0].name
        if alloc.kind == "ExternalInput":
            if name != partition_name:
                in_names.append(name)
        elif alloc.kind == "ExternalOutput":
            shape = tuple(alloc.tensor_shape)
            dtype = mybir.dt.np(alloc.dtype)
            out_names.append(name)
            out_avals.append(jax.core.ShapedArray(shape, dtype))
            zero_outs.append(np.zeros(shape, dtype))
    n_params = len(in_names)
    all_in_names = list(in_names) + list(out_names)
    if partition_name is not None:
        all_in_names.append(partition_name)

    chain_in_idx = in_names.index("chain") if "chain" in in_names else None
    chain_out_idx = (out_names.index("chout")
                     if "chout" in out_names else None)

    def _one(args_list):
        operands = list(args_list)
        if partition_name is not None:
            operands.append(partition_id_tensor())
        return _bass_exec_p.bind(
            *operands, out_avals=tuple(out_avals),
            in_names=tuple(all_in_names), out_names=tuple(out_names),
            lowering_input_output_aliases=(), sim_require_finite=True,
            sim_require_nnan=True, nc=nc)

    def _body(*args):
        return tuple(_one(list(args)))

    devices = jax.devices()[:n_cores]
    mesh = Mesh(np.asarray(devices), ("core",))
    nio = n_params + len(out_names)
    sharded1 = jax.jit(
        shard_map(_body, mesh=mesh,
                  in_specs=(PartitionSpec("core"),) * nio,
                  out_specs=(PartitionSpec("core"),) * len(out_names),
                  check_rep=False),
        keep_unused=True)

    concat_in = [np.concatenate([np.asarray(in_maps[c][nm])
                                 for c in range(n_cores)], axis=0)
                 for nm in in_names]
    sh = NamedSharding(mesh, PartitionSpec("core"))
    dev_in = [jax.device_put(a, sh) for a in concat_in]
    zs = [jax.device_put(
        np.zeros((n_cores * z.shape[0], *z.shape[1:]), z.dtype), sh)
        for z in zero_outs]

    out_arrs = jax.block_until_ready(sharded1(*dev_in, *zs))
    best_ns = None
    if chain_in_idx is not None and reps > 1:
        # Async-dispatch a chain of D executions (each consuming the previous
        # chain output so the device must serialize them), measured against a
        # depth-1 chain to cancel dispatch/RPC overhead.
        def run_chain(depth):
            t0 = time.perf_counter()
            args = list(dev_in)
            outs = None
            for _ in range(depth):
                outs = sharded1(*args, *zs)
                args[chain_in_idx] = outs[chain_out_idx]
            jax.block_until_ready(outs)
            return time.perf_counter() - t0
        depth = int(os.environ.get("KERNEL_CHAIN", "24"))
        run_chain(2)  # warm
        w1 = min(run_chain(1) for _ in range(reps))
        wD = min(run_chain(depth) for _ in range(max(2, reps // 2)))
        if wD > w1:
            best_ns = int((wD - w1) / (depth - 1) * 1e9)
    results = [{nm: np.asarray(out_arrs[i]).reshape(
        n_cores, *out_avals[i].shape)[c]
        for i, nm in enumerate(out_names)} for c in range(n_cores)]
    return results, best_ns


def kernel(Y, P, N, K1, b1, Wc0, bc0, Wc1, bc1, Wc2, bc2,
           Wi0, bi0, Wi1, bi1, Wi2, bi2, Wf, bf, Wo, bo, Wa,
           Wt, bt, Wd1, bd1, Wd2, bd2):
    global _LAST_HW_NS
    import concourse.bass as bass_mod
    import concourse.tile as tile_mod
    from concourse import bacc as bacc_mod, mybir
    from concourse.bass_utils import run_bass_kernel_spmd

    f32 = np.float32
    Y = np.asarray(Y, f32)
    Pa = np.asarray(P, f32)
    Na = np.asarray(N, f32)

    if "nc" not in _CACHE:
        _CACHE["nc"] = _build(bass_mod, tile_mod, bacc_mod, mybir)
    nc = _CACHE["nc"]

    wmap = _fold_weights(K1, b1, Wc0, bc0, Wc1, bc1, Wc2, bc2,
                         Wi0, bi0, Wi1, bi1, Wi2, bi2, Wf, bf, Wo, bo, Wa,
                         Wt, bt, Wd1, bd1, Wd2, bd2)

    series = np.concatenate([Y, Pa, Na], axis=2)        # [B, T, 21]
    xT = series.transpose(1, 2, 0).astype(np.float16)   # [T, 21, B]

    in_maps = []
    for c in range(NCORES):
        m = dict(wmap)
        m["chain"] = np.zeros((1, 4), f32)
        m["xT"] = np.ascontiguousarray(
            xT[:, :, c * BC:(c + 1) * BC].reshape(T, S * BC))
        in_maps.append(m)

    if os.environ.get("KERNEL_TIME", "1") == "1":
        results, best_ns = _timed_spmd(nc, in_maps,
                                       reps=int(os.environ.get("KERNEL_REPS",
                                                               "6")))
        if best_ns:
            _LAST_HW_NS = best_ns
    else:
        res = run_bass_kernel_spmd(nc, in_maps, list(range(NCORES)))
        results = res.results
    _CACHE["res0"] = results[0]
    out = np.concatenate([results[c]["out"].reshape(BC, 1)
                          for c in range(NCORES)], axis=0)
    return out.astype(f32)


# revision 89
# speedup vs baseline: 1.5907x; 1.5907x over previous
"""MI-LSTM full-forward Trainium2 kernel (8 NeuronCores, data-parallel batch).

Entire model runs on device per core (batch shard of 256):
  stage 1: 21 shared-weight scalar-input LSTMs, fused over (series, batch)
           = 5376 sequences/core, 50 steps, state transposed [H, seq].
  stage 2: MI-LSTM with 3-branch input attention, x-projections fused into
           the stage-1 step (block matmuls over the 21 series' h outputs).
  stage 3: temporal attention over T + dense head, via PE transposes into a
           [batch, (t, h)] layout so softmax-over-T is free-dim math.

Numerics: fp16 data / fp32 psum. All sigmoids are tanh-basis
(sig(x) = (tanh(x/2)+1)/2) so every activation instr is Tanh(scale=0.5)
and Exp stays in the same ACT table set. Doubling folds (h stored as 2h,
c as 2c) keep the +1 shifts free; compensating 1/2 factors are folded
into the weights host-side.
"""
import os
import sys

sys.path.insert(0, "/opt/trn_rl_repo")

import numpy as np

H = 64
NS = 10
S = 21
B = 2048
T = 50
NCORES = 8
BC = B // NCORES          # 256
N = S * BC                # 5376 sequences per core
HN = N // 2               # 2688
DP = NS * H

_CACHE = {}
_LAST_HW_NS = None


def _build(nc_mod, tile_mod, bacc_mod, mybir):
    f16 = mybir.dt.float16
    f32 = mybir.dt.float32
    AX = mybir.AxisListType
    OP = mybir.AluOpType
    AF = mybir.ActivationFunctionType
    from contextlib import ExitStack

    nc = bacc_mod.Bacc("TRN2", target_bir_lowering=False, debug=False,
                       num_devices=NCORES)

    dram = {}
    def din(name, shape, dt=f16):
        dram[name] = nc.dram_tensor(name, shape, dt, kind="ExternalInput").ap()
    din("xT", [T, N])
    din("k1e", [66, 256])
    din("wzx", [66, 22 * 128])
    din("whh", [64, 512])
    din("waa", [64, 64])
    din("wtb50", [128, T * 64])
    din("idn", [128, 128])
    din("wd1", [64, 64])
    din("wd2", [64, 1])
    din("bd1", [64, 1], f32)
    din("btp", [128, 1], f32)
    din("dk", [3, 192])
    din("cst", [1, 4], f32)   # [bt, bd2, unused, unused]
    din("chain", [1, 4], f32)
    out_ap = nc.dram_tensor("out", [1, BC], f32, kind="ExternalOutput").ap()
    chout_ap = nc.dram_tensor("chout", [1, 4], f32, kind="ExternalOutput").ap()
    DBG = os.environ.get("KERNEL_DEBUG", "0") == "1"
    dbg = {}
    if DBG:
        for nm, shp in [("d_g1", [128, N]), ("d_h0", [64, N]),
                        ("d_cc0", [128, HN]), ("d_tt0", [128, 1024]),
                        ("d_e1", [3, BC]), ("d_l2c1", [64, BC]),
                        ("d_h2all", [64, T * BC]), ("d_std", [128, T * 64]),
                        ("d_ctxT", [64, 256]), ("d_lg1", [64, 768]),
                        ("d_gs31", [64, 768]), ("d_lt1", [64, 768]),
                        ("d_tt1", [128, 1024]), ("d_gst1", [64, BC]),
                        ("d_cc2in1", [64, BC]), ("d_h1", [64, N])]:
            dbg[nm] = nc.dram_tensor(nm, shp, mybir.dt.float16
                                     if nm != "d_ctxT" else mybir.dt.float16,
                                     kind="ExternalOutput").ap()

    with tile_mod.TileContext(nc) as tc:
        with ExitStack() as ctx:
            wp = ctx.enter_context(tc.tile_pool(name="wp", bufs=1))
            sp = ctx.enter_context(tc.tile_pool(name="sp", bufs=1))
            ps1 = ctx.enter_context(tc.tile_pool(name="ps1", bufs=2, space="PSUM"))
            ps2 = ctx.enter_context(tc.tile_pool(name="ps2", bufs=1, space="PSUM"))

            # ---- weights ----
            k1e = wp.tile([66, 256], f16)
            wzx = wp.tile([66, 22 * 128], f16)
            whh = wp.tile([64, 512], f16)
            waa = wp.tile([64, 64], f16)
            wtb50 = wp.tile([128, T * 64], f16)
            idn = wp.tile([128, 128], f16)
            wd1 = wp.tile([64, 64], f16)
            wd2 = wp.tile([64, 1], f16)
            bd1 = wp.tile([64, 1], f32)
            btp = wp.tile([128, 1], f32)
            dk = wp.tile([3, 192], f16)
            cst = wp.tile([1, 4], f32)
            for nm, t_ in [("k1e", k1e), ("wzx", wzx), ("whh", whh),
                           ("waa", waa), ("wtb50", wtb50), ("idn", idn),
                           ("wd1", wd1), ("wd2", wd2), ("bd1", bd1),
                           ("btp", btp), ("dk", dk), ("cst", cst)]:
                nc.sync.dma_start(t_[:], dram[nm])

            # small constant lhsTs built on device
            onesk = wp.tile([64, 9], f16)      # 3 blocks of [64,3], col k of block k = 1
            nc.vector.memset(onesk[:], 0.0)
            for k in range(3):
                nc.vector.memset(onesk[:, 3 * k + k: 3 * k + k + 1], 1.0)
            ones3 = wp.tile([3, 1], f16)
            nc.vector.memset(ones3[:], 1.0)
            ones1 = wp.tile([1, 64], f16)
            nc.vector.memset(ones1[:], 1.0)
            ones1b = wp.tile([1, 128], f16)
            nc.vector.memset(ones1b[:], 1.0)
            bneg = wp.tile([3, 1], f32)
            nc.vector.memset(bneg[:], -2.0)

            # ---- state tiles ----
            # hx rows: [H (0:64); ones (64); x_t (65)]
            hx = [sp.tile([66, N], f16, name=f"hx{i}") for i in range(2)]
            for i in range(2):
                nc.vector.memset(hx[i][0:64, :], 0.0)
                nc.vector.memset(hx[i][64:65, :], 1.0)
            nc.sync.dma_start(hx[0][65:66, :], dram["xT"][0:1, :])

            ccp = [sp.tile([128, HN], f16, name=f"ccp{i}") for i in range(2)]
            nc.vector.memset(ccp[0][:], 0.0)
            cc2 = [sp.tile([64, BC], f16, name=f"cc2{i}") for i in range(2)]
            nc.vector.memset(cc2[0][:], 0.0)
            gst = sp.tile([64, BC], f16)
            gs3 = sp.tile([64, 768], f16)
            nc.vector.memset(gs3[:], 0.0)

            gall = sp.tile([128, 2 * N], f16)
            g1f = sp.tile([128, HN], f16)
            g1i = sp.tile([128, HN], f16)
            g2o = sp.tile([128, HN], f16)
            g2j = sp.tile([128, HN], f16)
            ta = sp.tile([128, HN], f16)
            tb = sp.tile([128, HN], f16)
            thp = sp.tile([128, HN], f16)

            h2all = sp.tile([64, T * BC], f16)
            h2stdA = sp.tile([128, T * 64], f16)
            h2stdB = sp.tile([128, T * 64], f16)
            ttile = sp.tile([128, 1024], f16)
            ttp = sp.tile([128, 768], f16)
            ltile = sp.tile([64, 768], f16)
            wtl = sp.tile([128, 1024], f16)
            lg = sp.tile([64, 768], f16)
            etile = sp.tile([3, BC], f16)
            rt = sp.tile([1, BC], f16)
            l2c = sp.tile([64, BC], f16)
            tf1 = sp.tile([64, BC], f16)
            o1 = sp.tile([64, BC], f16)
            m2 = sp.tile([64, BC], f16)
            th2 = sp.tile([64, BC], f16)

            CH = [(0, 1024), (1024, 1024), (2048, 1024), (3072, 1024),
                  (4096, 1024), (5120, 256)]

            for t in range(T):
                cur, nxt = hx[t % 2], hx[(t + 1) % 2]
                ccc, ccn = ccp[t % 2], ccp[(t + 1) % 2]
                c2c, c2n = cc2[t % 2], cc2[(t + 1) % 2]

                if t + 1 < T:
                    nc.sync.dma_start(nxt[65:66, :], dram["xT"][t + 1:t + 2, :])

                # ---- stage 1: gates ----
                # gall cols 0:N = M0 ([f|i]), N:2N = M1 ([o|j]); one tanh act
                # per psum chunk (scale/bias folds live in the weights).
                # M0 first so the f/i packs and the TB product (pool) can
                # start while M1 chunks are still in the matmul/act pipe.
                for m in (0, 1):
                    for c0, cw in CH:
                        ps = ps1.tile([128, 1024], f32, tag="s1", name="s1ps")
                        for s0 in range(0, cw, 512):
                            sw = min(512, cw - s0)
                            nc.tensor.matmul(
                                ps[:, s0:s0 + sw],
                                k1e[:, m * 128:(m + 1) * 128],
                                cur[0:66, c0 + s0:c0 + s0 + sw],
                                start=True, stop=True)
                        nc.scalar.activation(
                            gall[:, N * m + c0:N * m + c0 + cw], ps[:, 0:cw],
                            AF.Tanh, bias=0.0, scale=0.5)

                # ---- stage 1: elementwise (fp16, packed halves) ----
                # gall cols 0:N = M0 tile [tf|ti], cols N:2N = M1 tile [to|tj]
                for hh in (0, 1):
                    for qq in (0, 1):
                        a = HN * hh + (HN // 2) * qq
                        b = a + HN // 2
                        sl = slice(64 * hh, 64 * (hh + 1))
                        cs = slice((HN // 2) * qq, (HN // 2) * (qq + 1))
                        nc.vector.tensor_scalar(g1f[sl, cs], gall[0:64, a:b],
                                                1.0, 0.5, OP.add, OP.mult)
                        nc.vector.tensor_scalar(g1i[sl, cs], gall[64:128, a:b],
                                                1.0, None, OP.add)
                        nc.vector.tensor_scalar(g2o[sl, cs],
                                                gall[0:64, N + a:N + b],
                                                1.0, None, OP.add)
                        nc.vector.tensor_scalar(g2j[sl, cs],
                                                gall[64:128, N + a:N + b],
                                                1.0, None, OP.mult)
                QW = HN // 4
                for q in range(4):
                    cs = slice(QW * q, QW * (q + 1))
                    nc.vector.tensor_tensor(ta[:, cs], g1i[:, cs], g2j[:, cs],
                                            OP.mult)
                    nc.vector.tensor_tensor(tb[:, cs], g1f[:, cs], ccc[:, cs],
                                            OP.mult)
                    nc.vector.tensor_tensor(ccn[:, cs], ta[:, cs], tb[:, cs],
                                            OP.add)
                    nc.scalar.activation(thp[:, cs], ccn[:, cs], AF.Tanh,
                                         bias=0.0, scale=0.5)
                    for hh in (0, 1):
                        sl = slice(64 * hh, 64 * (hh + 1))
                        nc.vector.tensor_tensor(
                            nxt[0:64, HN * hh + QW * q:HN * hh + QW * (q + 1)],
                            g2o[sl, cs], thp[sl, cs], OP.mult)
                if DBG and t == 0:
                    nc.sync.dma_start(dbg["d_g1"], gall[:, 0:N])
                    nc.sync.dma_start(dbg["d_h0"], nxt[0:64, :])
                    nc.sync.dma_start(dbg["d_cc0"], ccn[:])

                # ---- stage 2: x-projections + recurrent into psum main ----
                pm = ps2.tile([128, 1024], f32, tag="s2m", name="s2m")
                # region A [0:256] = [i1|C1] over P series s=1..10
                # region B [256:512] = [i2|C2] over N series s=11..20
                # region C [512:768] = [i0|C0] over series 0
                # region D [768:1024] = [o|f] over series 0
                # each region's accumulation group must close before the next
                # region in the same psum bank opens (start zeroes the bank's
                # pending-group region)
                hprev = h2all[:, BC * (t - 1):BC * t] if t > 0 else None
                blk = 0
                for reg, srs in ((0, range(1, 11)), (1, range(11, 21)),
                                 (2, range(0, 1)), (3, range(0, 1))):
                    nsr = len(srs)
                    for j, s in enumerate(srs):
                        nc.tensor.matmul(
                            pm[:, 256 * reg:256 * (reg + 1)],
                            wzx[0:65, 128 * blk:128 * (blk + 1)],
                            nxt[0:65, BC * s:BC * (s + 1)],
                            start=(j == 0), stop=(j == nsr - 1 and t == 0))
                        blk += 1
                    if t > 0:
                        nc.tensor.matmul(pm[:, 256 * reg:256 * (reg + 1)],
                                         whh[:, 128 * reg:128 * (reg + 1)],
                                         hprev, start=False, stop=True)

                pa = ps2.tile([128, 512], f32, tag="smA", name="smA")
                if t > 0:
                    nc.tensor.matmul(pa[64:128, 0:BC], waa[:], c2c[:],
                                     start=True, stop=True)

                nc.scalar.activation(ttile[:], pm[:], AF.Tanh, bias=0.0,
                                     scale=0.5)
                if t > 0:
                    nc.scalar.activation(gst[:], pa[64:128, 0:BC], AF.Tanh,
                                         bias=0.0, scale=0.5)
                    for k in range(3):
                        nc.vector.tensor_scalar(gs3[:, 256 * k:256 * (k + 1)],
                                                gst[:], 1.0, None, OP.mult)

                # l_k = (ti_k+1)*tC_k ; blocks [l1 | l2 | l0]
                nc.vector.tensor_scalar(ttp[64:128, :], ttile[0:64, 0:768],
                                        1.0, None, OP.add)
                nc.vector.tensor_tensor(ltile[:], ttp[64:128, :],
                                        ttile[64:128, 0:768], OP.mult)
                nc.vector.tensor_tensor(lg[:], ltile[:], gs3[:], OP.mult)

                # u_k into psum rows 0:3 via sparse ones-column lhsTs
                for k in range(3):
                    nc.tensor.matmul(pa[0:3, 0:BC], onesk[:, 3 * k:3 * k + 3],
                                     lg[:, BC * k:BC * (k + 1)],
                                     start=(k == 0), stop=(k == 2),
                                     skip_group_check=True)
                nc.scalar.activation(etile[:], pa[0:3, 0:BC], AF.Exp,
                                     bias=bneg[:], scale=0.5)
                nc.tensor.matmul(pa[32:33, 256:512], ones3[:], etile[:],
                                 start=True, stop=True)
                with nc.allow_low_precision(reason="softmax denom fp16"):
                    nc.vector.reciprocal(rt[:], pa[32:33, 256:512])
                nc.tensor.matmul(pa[64:128, 256:512], ones1[:], rt[:],
                                 start=True, stop=True)
                pb = ps2.tile([128, 512], f32, tag="smB", name="smB")
                nc.tensor.matmul(pb[0:64, 0:256], dk[:, 0:64], etile[:],
                                 start=True, stop=True)
                nc.tensor.matmul(pb[0:64, 256:512], dk[:, 64:128], etile[:],
                                 start=True, stop=True)
                nc.tensor.matmul(pa[0:64, 256:512], dk[:, 128:192], etile[:],
                                 start=True, stop=True)
                # W = sum_k eB_k * l_k  (rows 64:127 so bases pair with rB)
                nc.vector.tensor_tensor(wtl[64:128, 0:256], pb[0:64, 0:256],
                                        ltile[:, 0:256], OP.mult)
                nc.vector.tensor_tensor(wtl[64:128, 256:512], pb[0:64, 256:512],
                                        ltile[:, 256:512], OP.mult)
                nc.vector.tensor_tensor(wtl[64:128, 512:768], pa[0:64, 256:512],
                                        ltile[:, 512:768], OP.mult)
                # (eB/rB reads stay on DVE: gpsimd cannot access PSUM)
                nc.vector.tensor_tensor(wtl[64:128, 768:1024],
                                        wtl[64:128, 0:256],
                                        wtl[64:128, 256:512], OP.add)
                nc.vector.tensor_tensor(wtl[64:128, 0:256],
                                        wtl[64:128, 768:1024],
                                        wtl[64:128, 512:768], OP.add)
                nc.vector.tensor_tensor(l2c[:], wtl[64:128, 0:256],
                                        pa[64:128, 256:512], OP.mult)
                if DBG and t == 0:
                    nc.sync.dma_start(dbg["d_tt0"], ttile[:])
                if DBG and t == 1:
                    nc.sync.dma_start(dbg["d_e1"], etile[:])
                    nc.sync.dma_start(dbg["d_l2c1"], l2c[:])
                    nc.sync.dma_start(dbg["d_lg1"], lg[:])
                    nc.sync.dma_start(dbg["d_gs31"], gs3[:])
                    nc.sync.dma_start(dbg["d_lt1"], ltile[:])
                    nc.sync.dma_start(dbg["d_tt1"], ttile[:])
                    nc.sync.dma_start(dbg["d_gst1"], gst[:])
                    nc.sync.dma_start(dbg["d_cc2in1"], c2c[:])
                    nc.sync.dma_start(dbg["d_h1"], nxt[0:64, :])

                # cell + output gate
                nc.vector.tensor_scalar(tf1[:], ttile[64:128, 768:1024], 1.0,
                                        0.5, OP.add, OP.mult)
                nc.vector.tensor_tensor(m2[:], tf1[:], c2c[:], OP.mult)
                nc.vector.tensor_tensor(c2n[:], m2[:], l2c[:], OP.add)
                nc.scalar.activation(th2[:], c2n[:], AF.Tanh, bias=0.0,
                                     scale=0.5)
                nc.vector.tensor_scalar(o1[:], ttile[0:64, 768:1024], 1.0,
                                        None, OP.add)
                nc.vector.tensor_tensor(h2all[:, BC * t:BC * (t + 1)], o1[:],
                                        th2[:], OP.mult)

                # transpose h2_t into [b, (t, h)] std layout
                pt = ps2.tile([128, 1024], f16, tag="smB", name="tp")
                for hh, dst in ((0, h2stdA), (1, h2stdB)):
                    nc.tensor.transpose(
                        pt[:, 64 * hh:64 * (hh + 1)],
                        h2all[:, BC * t + 128 * hh:BC * t + 128 * (hh + 1)],
                        idn[0:64, 0:64])
                    nc.vector.tensor_scalar(
                        dst[:, 64 * t:64 * (t + 1)],
                        pt[:, 64 * hh:64 * (hh + 1)],
                        1.0, None, OP.mult)

            if DBG:
                nc.sync.dma_start(dbg["d_h2all"], h2all[:])
                nc.sync.dma_start(dbg["d_std"], h2stdA[:])

            # ---- stage 3: temporal attention + dense head ----
            prodE = sp.tile([128, T * 64], f16)
            ctxp = sp.tile([128, T * 64], f16)
            ctx16 = sp.tile([128, 64], f16)
            ctxT = sp.tile([64, 2 * 128], f16)
            outs = sp.tile([1, BC], f32)
            for hh, std in ((0, h2stdA), (1, h2stdB)):
                ered = sp.tile([128, T], f32, name=f"ered{hh}")
                eh = sp.tile([128, T], f16, name=f"eh{hh}")
                ex = sp.tile([128, T], f16, name=f"ex{hh}")
                srd = sp.tile([128, 1], f32, name=f"srd{hh}")
                rrd = sp.tile([128, 1], f32, name=f"rrd{hh}")
                beta = sp.tile([128, T], f32, name=f"beta{hh}")
                ctxr = sp.tile([128, 64], f32, name=f"ctxr{hh}")
                nc.vector.tensor_tensor(prodE[:], std[:], wtb50[:], OP.mult)
                nc.vector.tensor_reduce(
                    ered[:], prodE[:].rearrange("p (t h) -> p t h", t=T),
                    AX.X, OP.add)
                nc.scalar.activation(eh[:], ered[:], AF.Tanh, bias=btp[:],
                                     scale=0.5)
                nc.scalar.activation(ex[:], eh[:], AF.Exp, bias=0.0, scale=1.0)
                nc.vector.tensor_reduce(srd[:], ex[:], AX.X, OP.add)
                nc.vector.reciprocal(rrd[:], srd[:])
                nc.vector.tensor_scalar(beta[:], ex[:], rrd[:], None, OP.mult)
                for tt_ in range(T):
                    nc.vector.tensor_scalar(
                        ctxp[:, 64 * tt_:64 * (tt_ + 1)],
                        std[:, 64 * tt_:64 * (tt_ + 1)],
                        beta[:, tt_:tt_ + 1], None, OP.mult)
                nc.vector.tensor_reduce(
                    ctxr[:], ctxp[:].rearrange("p (t h) -> p h t", t=T),
                    AX.X, OP.add)
                nc.vector.tensor_scalar(ctx16[:], ctxr[:], 1.0, None, OP.mult)
                ptc = ps2.tile([128, 1024], f16, tag="smB", name=f"ptc{hh}")
                nc.tensor.transpose(ptc[0:64, 0:128], ctx16[:], idn[:])
                nc.vector.tensor_scalar(ctxT[:, 128 * hh:128 * (hh + 1)],
                                        ptc[0:64, 0:128],
                                        1.0, None, OP.mult)
            if DBG:
                nc.sync.dma_start(dbg["d_ctxT"], ctxT[:])
            pz = ps2.tile([128, 1024], f32, tag="s2m", name="pz")
            nc.tensor.matmul(pz[0:64, 0:256], wd1[:], ctxT[:],
                             start=True, stop=True)
            r1 = sp.tile([64, BC], f16)
            nc.scalar.activation(r1[:], pz[0:64, 0:256], AF.Relu,
                                 bias=bd1[:], scale=1.0)
            nc.tensor.matmul(pz[0:1, 512:768], wd2[:], r1[:],
                             start=True, stop=True)
            nc.vector.tensor_scalar(outs[:], pz[0:1, 512:768],
                                    cst[0:1, 1:2], None, OP.add)
            nc.sync.dma_start(out_ap, outs[:])
            chs = sp.tile([1, 4], f32)
            nc.sync.dma_start(chs[:], dram["chain"])
            nc.sync.dma_start(chout_ap, chs[:])

    nc.compile()
    return nc


def _fold_weights(K1, b1, Wc0, bc0, Wc1, bc1, Wc2, bc2,
                  Wi0, bi0, Wi1, bi1, Wi2, bi2, Wf, bf, Wo, bo, Wa,
                  Wt, bt, Wd1, bd1, Wd2, bd2):
    f32 = np.float32
    # ---- stage-1 K1e [66, 256]: device gate order [f | i | o | j] ----
    K1 = np.asarray(K1, f32)
    b1 = np.asarray(b1, f32)
    kx, kh = K1[0], K1[1:]          # [256], [64, 256]
    gi, gj, gf, go = (slice(0, 64), slice(64, 128), slice(128, 192),
                      slice(192, 256))
    # rows: [Kh/2 (0:64); bias (64, pairs with ones row); Kx (65, x row)]
    k1e = np.zeros((66, 256), f32)
    for dcol, scol in ((0, gf), (64, gi), (128, go), (192, gj)):
        k1e[0:64, dcol:dcol + 64] = kh[:, scol] * 0.5   # h stored doubled
        k1e[64, dcol:dcol + 64] = b1[scol]
        k1e[65, dcol:dcol + 64] = kx[scol]
    k1e[64, 0:64] += 1.0                                 # forget bias
    k1e[:, 192:256] *= 2.0                               # j: tanh scale fold

    # ---- stage-2 zx blocks ----
    defs = {
        "i0": (np.asarray(Wi0, f32), np.asarray(bi0, f32), 0, H, 1.0),
        "i1": (np.asarray(Wi1, f32), np.asarray(bi1, f32), H, DP, 1.0),
        "i2": (np.asarray(Wi2, f32), np.asarray(bi2, f32), H + DP, DP, 1.0),
        "C0": (np.asarray(Wc0, f32), np.asarray(bc0, f32), 0, H, 2.0),
        "C1": (np.asarray(Wc1, f32), np.asarray(bc1, f32), H, DP, 2.0),
        "C2": (np.asarray(Wc2, f32), np.asarray(bc2, f32), H + DP, DP, 2.0),
        "f": (np.asarray(Wf, f32), np.asarray(bf, f32), 0, H, 1.0),
        "o": (np.asarray(Wo, f32), np.asarray(bo, f32), 0, H, 1.0),
    }
    # X feature s-block for gate g at series s: rows (s_rel*64:(s_rel+1)*64)
    # wzx blocks: A: s=1..10 [i1|C1]; B: s=11..20 [i2|C2]; C: s=0 [i0|C0];
    # D: s=0 [o|f].  lhsT [66, 128]: rows 0:64 h-part (/2), row 64 = 0,
    # row 65 = bias (first block of each region only, scaled like gate).
    wzx = np.zeros((66, 22 * 128), f32)
    blk = 0
    regions = [("i1", "C1", list(range(1, 11))),
               ("i2", "C2", list(range(11, 21))),
               ("i0", "C0", [0]), ("o", "f", [0])]
    for gl, gr, srs in regions:
        for j, s in enumerate(srs):
            for half, gname in ((0, gl), (1, gr)):
                Wg, bg, x0, xd, sc = defs[gname]
                s_rel = s - (1 if gname in ("i1", "C1") else
                             11 if gname in ("i2", "C2") else 0)
                rows = Wg[s_rel * 64: s_rel * 64 + 64]  # x-part block [64, 64]
                wzx[0:64, 128 * blk + 64 * half:128 * blk + 64 * (half + 1)] \
                    = rows * 0.5 * sc
                if j == 0:
                    wzx[64, 128 * blk + 64 * half:
                        128 * blk + 64 * (half + 1)] = bg * sc
            blk += 1

    whh = np.zeros((64, 512), f32)
    for reg, (gl, gr) in enumerate((("i1", "C1"), ("i2", "C2"),
                                    ("i0", "C0"), ("o", "f"))):
        for half, gname in ((0, gl), (1, gr)):
            Wg, bg, x0, xd, sc = defs[gname]
            whh[:, 128 * reg + 64 * half:128 * reg + 64 * (half + 1)] = \
                Wg[xd:] * 0.5 * sc

    waa = np.asarray(Wa, f32)                           # unscaled: CC2 = 2c
    wtb50 = np.tile(np.asarray(Wt, f32).reshape(1, 1, 64),
                    (128, T, 1)).reshape(128, T * 64)
    wd1 = np.asarray(Wd1, f32) * 0.5                    # ctx doubled
    wd2 = np.asarray(Wd2, f32).reshape(64, 1)
    bd1v = np.asarray(bd1, f32).reshape(64, 1)
    btf = float(np.asarray(bt).reshape(-1)[0])
    cstv = np.array([[btf, float(np.asarray(bd2).reshape(-1)[0]), 0.0, 0.0]],
                    f32)
    f16 = np.float16
    return dict(k1e=k1e.astype(f16), wzx=wzx.astype(f16),
                whh=whh.astype(f16), waa=waa.astype(f16),
                wtb50=wtb50.astype(f16),
                idn=np.eye(128, dtype=f16), wd1=wd1.astype(f16),
                wd2=wd2.astype(f16), bd1=bd1v,
                btp=np.full((128, 1), btf, f32),
                dk=np.kron(np.eye(3), np.ones((1, 64))).astype(f16), cst=cstv)


def _timed_spmd(nc, in_maps, reps=6):
    """Replicate bass2jax.run_bass_via_pjrt's multi-core path, but keep the
    compiled callable and device-resident inputs so repeated executions
    measure steady-state device time (dispatch overhead included)."""
    import time
    import jax
    import numpy as jnp_np
    from jax.sharding import Mesh, PartitionSpec, NamedSharding
    from jax.experimental.shard_map import shard_map
    from concourse import mybir
    from concourse.bass2jax import (install_neuronx_cc_hook, _bass_exec_p,
                                    partition_id_tensor)

    install_neuronx_cc_hook()
    n_cores = len(in_maps)
    partition_name = (nc.partition_id_tensor.name
                      if nc.partition_id_tensor else None)
    in_names, out_names, out_avals, zero_outs = [], [], [], []
    for alloc in nc.m.functions[0].allocations:
        if not isinstance(alloc, mybir.MemoryLocationSet):
            continue
        name = alloc.memorylocations[0].name
        if alloc.kind == "ExternalInput":
            if name != partition_name:
                in_names.append(name)
        elif alloc.kind == "ExternalOutput":
            shape = tuple(alloc.tensor_shape)
            dtype = mybir.dt.np(alloc.dtype)
            out_names.append(name)
            out_avals.append(jax.core.ShapedArray(shape, dtype))
            zero_outs.append(np.zeros(shape, dtype))
    n_params = len(in_names)
    all_in_names = list(in_names) + list(out_names)
    if partition_name is not None:
        all_in_names.append(partition_name)

    chain_in_idx = in_names.index("chain") if "chain" in in_names else None
    chain_out_idx = (out_names.index("chout")
                     if "chout" in out_names else None)

    def _one(args_list):
        operands = list(args_list)
        if partition_name is not None:
            operands.append(partition_id_tensor())
        return _bass_exec_p.bind(
            *operands, out_avals=tuple(out_avals),
            in_names=tuple(all_in_names), out_names=tuple(out_names),
            lowering_input_output_aliases=(), sim_require_finite=True,
            sim_require_nnan=True, nc=nc)

    def _body(*args):
        return tuple(_one(list(args)))

    devices = jax.devices()[:n_cores]
    mesh = Mesh(np.asarray(devices), ("core",))
    nio = n_params + len(out_names)
    sharded1 = jax.jit(
        shard_map(_body, mesh=mesh,
                  in_specs=(PartitionSpec("core"),) * nio,
                  out_specs=(PartitionSpec("core"),) * len(out_names),
                  check_rep=False),
        keep_unused=True)

    concat_in = [np.concatenate([np.asarray(in_maps[c][nm])
                                 for c in range(n_cores)], axis=0)
                 for nm in in_names]
    sh = NamedSharding(mesh, PartitionSpec("core"))
    dev_in = [jax.device_put(a, sh) for a in concat_in]
    zs = [jax.device_put(
        np.zeros((n_cores * z.shape[0], *z.shape[1:]), z.dtype), sh)
        for z in zero_outs]

    out_arrs = jax.block_until_ready(sharded1(*dev_in, *zs))
    best_ns = None
    if chain_in_idx is not None and reps > 1:
        # Async-dispatch a chain of D executions (each consuming the previous
        # chain output so the device must serialize them), measured against a
        # depth-1 chain to cancel dispatch/RPC overhead.
        def run_chain(depth):
            t0 = time.perf_counter()
            args = list(dev_in)
            outs = None
            for _ in range(depth):
                outs = sharded1(*args, *zs)
                args[chain_in_idx] = outs[chain_out_idx]
            jax.block_until_ready(outs)
            return time.perf_counter() - t0
        depth = int(os.environ.get("KERNEL_CHAIN", "24"))
        run_chain(2)  # warm
        w1 = min(run_chain(1) for _ in range(reps))
        wD = min(run_chain(depth) for _ in range(max(2, reps // 2)))
        if wD > w1:
            best_ns = int((wD - w1) / (depth - 1) * 1e9)
    results = [{nm: np.asarray(out_arrs[i]).reshape(
        n_cores, *out_avals[i].shape)[c]
        for i, nm in enumerate(out_names)} for c in range(n_cores)]
    return results, best_ns


def kernel(Y, P, N, K1, b1, Wc0, bc0, Wc1, bc1, Wc2, bc2,
           Wi0, bi0, Wi1, bi1, Wi2, bi2, Wf, bf, Wo, bo, Wa,
           Wt, bt, Wd1, bd1, Wd2, bd2):
    global _LAST_HW_NS
    import concourse.bass as bass_mod
    import concourse.tile as tile_mod
    from concourse import bacc as bacc_mod, mybir
    from concourse.bass_utils import run_bass_kernel_spmd

    f32 = np.float32
    Y = np.asarray(Y, f32)
    Pa = np.asarray(P, f32)
    Na = np.asarray(N, f32)

    if "nc" not in _CACHE:
        _CACHE["nc"] = _build(bass_mod, tile_mod, bacc_mod, mybir)
    nc = _CACHE["nc"]

    wmap = _fold_weights(K1, b1, Wc0, bc0, Wc1, bc1, Wc2, bc2,
                         Wi0, bi0, Wi1, bi1, Wi2, bi2, Wf, bf, Wo, bo, Wa,
                         Wt, bt, Wd1, bd1, Wd2, bd2)

    series = np.concatenate([Y, Pa, Na], axis=2)        # [B, T, 21]
    xT = series.transpose(1, 2, 0).astype(np.float16)   # [T, 21, B]

    in_maps = []
    for c in range(NCORES):
        m = dict(wmap)
        m["chain"] = np.zeros((1, 4), f32)
        m["xT"] = np.ascontiguousarray(
            xT[:, :, c * BC:(c + 1) * BC].reshape(T, S * BC))
        in_maps.append(m)

    if os.environ.get("KERNEL_TIME", "1") == "1":
        results, best_ns = _timed_spmd(nc, in_maps,
                                       reps=int(os.environ.get("KERNEL_REPS",
                                                               "6")))
        if best_ns:
            _LAST_HW_NS = best_ns
    else:
        res = run_bass_kernel_spmd(nc, in_maps, list(range(NCORES)))
        results = res.results
    _CACHE["res0"] = results[0]
    out = np.concatenate([results[c]["out"].reshape(BC, 1)
                          for c in range(NCORES)], axis=0)
    return out.astype(f32)
